# revision 21
# baseline (speedup 1.0000x reference)
"""BottomRightPool (2D cummax) Trainium2 Bass kernel.

pool[b,c,i,j] = max(x[b,c,:i+1,:j+1])  ==  cummax over H, then over W.

Key identity: pool rows are non-decreasing along w, so
    pool[i, :] = scan_j ( state = max(state, x[i, j], pool[i-1, j]) )
because cummax_w(pool[i-1, :]) == pool[i-1, :].  tensor_tensor_scan computes
exactly  state = max(max(data0, state), data1), so ONE scan instruction per
row (data0 = x row i, data1 = pool row i-1) performs BOTH cummax passes.

Perf notes (all numbers measured on this HW via loop-slope):
  - All HBM traffic is bf16: tolerance is 2e-2 and bf16 rounding is ~2e-3
    (max() is exact in bf16, so error == input rounding). Halves DMA vs
    f32: 33.55 MB/core -> ~104 us at the measured 322 GB/s 8-core
    concurrent rate.
  - Scans must run on DVE (TensorScalarPtr is illegal on Pool for NC v3;
    Pool compute ops that do compile run ~20 us/instr — unusable). DVE
    scan throughput is ~2.2-2.4 ns/elem regardless of batching (big
    masked multi-row scans with a min-mask row reset are correct but no
    faster per element), so the fused one-scan-per-row form is optimal:
    512 scans/core x ~300 ns ~= 150 us, the kernel's bottleneck.
  - 4 lanes (one per 128-slice chunk) round-robin per row so adjacent DVE
    scans come from independent chains.
  - HB=16 with deep tile pools (5 generations) pipelines DMA fully under
    the scans; in-DMAs issue on SP, out-DMAs alternate Act/Pool so no
    single sequencer's DGE time (~0.6 us/DMA) becomes a serial tail.
  - Net ~177 us/core: scan-rate-bound plus ~20% scan slowdown from
    SBUF port contention with concurrent DMA streams.
"""

import numpy as np

N_CORES = 8
B, C, H, W = 16, 256, 128, 128
S = B * C                    # 4096 independent (b,c) slices
SPC = S // N_CORES           # 512 slices per core
CHUNK = 128                  # slices per tile (partition dim)
HB = 16                      # rows per h-block tile
NEG = -3.0e38

# Engine per lane (lane = slice chunk): "v" = DVE, "p" = GPSIMD/Pool.
# (Pool rejected: TensorScalarPtr is not a legal Pool opcode on NC v3.)
LANE_ENGINES = ("v", "v", "v", "v")


def _build_nc(repeat=None):
    """Build the per-core Bass program. repeat=None emits the plain kernel;
    repeat=R wraps the whole workload in a hardware For_i loop (benchmarking
    only — output is just rewritten R times)."""
    import concourse.mybir as mybir
    import concourse.tile as tile
    from concourse import bacc

    nc = bacc.Bacc(None, target_bir_lowering=False)
    DT = mybir.dt.bfloat16
    n_l = SPC // CHUNK
    # Host delivers x permuted to [p, g, H, W] (slice index = g*CHUNK + p)
    # so one DMA per h-block can gather all four lanes contiguously.
    xd = nc.dram_tensor("x", [CHUNK, n_l, H, W], DT, kind="ExternalInput")
    od = nc.dram_tensor("out", [CHUNK, n_l, H, W], DT, kind="ExternalOutput")
    MAX = mybir.AluOpType.max

    n_lanes = SPC // CHUNK
    assert n_lanes == len(LANE_ENGINES)

    with tile.TileContext(nc) as tc:
        with tc.tile_pool(name="ina", bufs=5) as pa, tc.tile_pool(
            name="outb", bufs=6
        ) as pb:

            def body():
                # One combined tile per h-block: partition p holds the rows of
                # slices {p, p+128, p+256, p+384} side by side in the free dim
                # (lane g occupies columns [g*HB*W, (g+1)*HB*W)). One DMA per
                # h-block per direction instead of four.
                prev = [None] * n_lanes  # pool row above current block
                for hb in range(H // HB):
                    h0 = hb * HB
                    A = pa.tile([CHUNK, n_lanes * HB * W], DT)
                    Bt = pb.tile([CHUNK, n_lanes * HB * W], DT)
                    nc.sync.dma_start(
                        out=A[:].rearrange(
                            "p (g h w) -> p g h w", g=n_lanes, h=HB
                        ),
                        in_=xd[:, :, h0 : h0 + HB],
                    )
                    for r in range(HB):
                        for lane in range(n_lanes):
                            base = (lane * HB + r) * W
                            row = slice(base, base + W)
                            if r == 0 and prev[lane] is None:
                                data1 = A[:, row]
                            elif r == 0:
                                data1 = prev[lane]
                            else:
                                data1 = Bt[:, base - W : base]
                            nc.vector.tensor_tensor_scan(
                                out=Bt[:, row],
                                data0=A[:, row],
                                data1=data1,
                                initial=NEG,
                                op0=MAX,
                                op1=MAX,
                            )
                    for lane in range(n_lanes):
                        base = (lane * HB + HB - 1) * W
                        prev[lane] = Bt[:, base : base + W]
                    out_eng = nc.scalar if hb % 2 == 0 else nc.gpsimd
                    out_eng.dma_start(
                        out=od[:, :, h0 : h0 + HB],
                        in_=Bt[:].rearrange(
                            "p (g h w) -> p g h w", g=n_lanes, h=HB
                        ),
                    )

            if repeat is None:
                body()
            else:
                with tc.For_i(0, repeat, 1):
                    body()
    nc.compile()
    return nc


def make_runner(nc, donate=True):
    """Compile once; return run(in_maps) plus the raw jitted callable.

    Mirrors concourse.bass2jax.run_bass_via_pjrt's multi-core path but keeps
    the jitted executable so repeated calls don't re-trace/re-compile.
    donate=False keeps passed device buffers alive so the bench can call the
    executable repeatedly with device-resident args (no host transfers).
    """
    import jax
    import concourse.mybir as mybir
    from jax.sharding import Mesh, PartitionSpec
    from jax.experimental.shard_map import shard_map
    from concourse.bass2jax import (
        _bass_exec_p,
        install_neuronx_cc_hook,
        partition_id_tensor,
    )

    install_neuronx_cc_hook()
    assert nc.dbg_addr is None
    partition_name = nc.partition_id_tensor.name if nc.partition_id_tensor else None

    in_names, out_names, out_avals, zero_outs = [], [], [], []
    for alloc in nc.m.functions[0].allocations:
        if not isinstance(alloc, mybir.MemoryLocationSet):
            continue
        name = alloc.memorylocations[0].name
        if alloc.kind == "ExternalInput":
            if name == partition_name:
                continue
            in_names.append(name)
        elif alloc.kind == "ExternalOutput":
            out_names.append(name)
            shape = tuple(alloc.tensor_shape)
            dtype = mybir.dt.np(alloc.dtype)
            out_avals.append(jax.core.ShapedArray(shape, dtype))
            zero_outs.append(np.zeros(shape, dtype))
    n_params = len(in_names)
    n_outs = len(out_avals)
    all_in_names = in_names + out_names
    if partition_name is not None:
        all_in_names = all_in_names + [partition_name]
    donate_idx = tuple(range(n_params, n_params + n_outs)) if donate else ()

    def _body(*args):
        operands = list(args)
        if partition_name is not None:
            operands.append(partition_id_tensor())
        outs = _bass_exec_p.bind(
            *operands,
            out_avals=tuple(out_avals),
            in_names=tuple(all_in_names),
            out_names=tuple(out_names),
            lowering_input_output_aliases=(),
            sim_require_finite=True,
            sim_require_nnan=True,
            nc=nc,
        )
        return tuple(outs)

    devices = jax.devices()[:N_CORES]
    mesh = Mesh(np.asarray(devices), ("core",))
    sharded = jax.jit(
        shard_map(
            _body,
            mesh=mesh,
            in_specs=(PartitionSpec("core"),) * (n_params + n_outs),
            out_specs=(PartitionSpec("core"),) * n_outs,
            check_rep=False,
        ),
        donate_argnums=donate_idx,
        keep_unused=True,
    )

    def make_args(in_maps):
        concat_in = [
            np.concatenate([np.asarray(m[name]) for m in in_maps], axis=0)
            for name in in_names
        ]
        concat_zeros = [
            np.zeros((N_CORES * z.shape[0], *z.shape[1:]), z.dtype)
            for z in zero_outs
        ]
        return concat_in + concat_zeros

    def run(in_maps):
        out_arrs = sharded(*make_args(in_maps))
        return [
            {
                name: np.asarray(out_arrs[i]).reshape(
                    N_CORES, *out_avals[i].shape
                )[c]
                for i, name in enumerate(out_names)
            }
            for c in range(N_CORES)
        ]

    return run, sharded, make_args


N_LANES = SPC // CHUNK


def _in_maps(xf: np.ndarray):
    """Shard the [S, H, W] input into per-core input dicts (bf16 on device),
    permuted to [p, g, H, W] per core for the combined-lane DMA layout."""
    import ml_dtypes

    xb = np.asarray(xf, dtype=ml_dtypes.bfloat16)
    out = []
    for k in range(N_CORES):
        shard = xb[k * SPC : (k + 1) * SPC].reshape(N_LANES, CHUNK, H, W)
        out.append({"x": np.ascontiguousarray(shard.transpose(1, 0, 2, 3))})
    return out


def _run(x: np.ndarray, trace: bool = False):
    """Returns (full_output, exec_time_ns_or_None)."""
    nc = _build_nc()
    run, _, _ = make_runner(nc)
    xf = np.ascontiguousarray(x, dtype=np.float32).reshape(S, H, W)
    in_maps = _in_maps(xf)
    results = run(in_maps)
    shards = [
        r["out"].transpose(1, 0, 2, 3).reshape(SPC, H, W) for r in results
    ]
    out = np.concatenate(shards, axis=0)
    return out.astype(np.float32).reshape(B, C, H, W), None


def kernel(x: np.ndarray) -> np.ndarray:
    return _run(x)[0]
